# revision 1
# baseline (speedup 1.0000x reference)
"""AriaGroupedGEMM (MoE grouped GEMM) on 8 TRN2 NeuronCores.

Problem: input [4096, 2048] f32, weight [8, 2048, 2048] f32,
tokens_per_expert [8] int32 (tokens pre-sorted by expert).
out[i] = input[i] @ weight[expert_of(i)].

Strategy: expert-parallel. Core g owns expert g's weight and its token
group (boundaries computed on host from tokens_per_expert). Each core
runs a dense [T_pad, 2048] @ [2048, 2048] GEMM in bf16 (fp32 PSUM
accumulation). Host pre-swizzles operands into SBUF-native layouts so
every DMA is fully contiguous, and gathers/unpads the result.

Schedule: all input DMAs go on the sync HWDGE ring in exact consumption
order (FIFO drain => just-in-time arrival). xt is chunked per m-tile and
the first two n-blocks' weights are chunked on k so the PE starts after
~1MB and never stalls once streaming. Warm-up matmuls on scratch data
lift the HAM clock gate before the real stream begins.
"""
import sys
import functools

for _p in ("/opt/trn_rl_repo", "/root/.axon_site/_ro/trn_rl_repo"):
    if _p not in sys.path:
        sys.path.insert(0, _p)

import numpy as np
import ml_dtypes

import concourse.mybir as mybir
import concourse.tile as tile
from concourse import bacc
from concourse import bass_utils

P = 128
K = 2048            # in_features (contraction)
N = 2048            # out_features
G = 8               # experts == cores
KO = K // P         # 16 k-subtiles
NB = N // 512       # 4 n-blocks of 512

COMPUTE_DT = mybir.dt.bfloat16
NP_COMPUTE = ml_dtypes.bfloat16
OUT_DT = mybir.dt.bfloat16      # psum(f32) -> bf16 on the way out; host upcasts

N_WARMUP_MM = 9     # N=512 warm-up matmuls (HAM ramp) before data lands
N_FILLER_MM = 0     # gap-filler matmuls inside the DMA-bound head phase


@functools.lru_cache(maxsize=4)
def _build(t_pad: int):
    """Build + compile the per-core GEMM graph for token-pad t_pad."""
    mt = t_pad // P  # m tiles of 128 tokens

    nc = bacc.Bacc("TRN2", target_bir_lowering=False, debug=False)

    # host-swizzled SBUF-native layouts (contiguous per partition line):
    # xt[mi, p, ko, j] = X[mi*P + j, ko*P + p]
    # w[p, ko, j]      = W[ko*P + p, j]
    xt_d = nc.dram_tensor(
        "xt", [mt, P, KO, P], COMPUTE_DT, kind="ExternalInput"
    ).ap()
    w_d = nc.dram_tensor(
        "w", [P, KO, N], COMPUTE_DT, kind="ExternalInput"
    ).ap()
    out_d = nc.dram_tensor("out", [t_pad, N], OUT_DT, kind="ExternalOutput").ap()

    # column blocks: two narrow head blocks shrink the DMA bytes gating the
    # first outputs; the tail blocks run at the efficient 512-wide MM rate
    if N == 2048:
        BLOCKS = [(0, 256), (256, 256), (512, 512), (1024, 512), (1536, 512)]
    else:
        BLOCKS = [(i * 512, 512) for i in range(N // 512)]
    NBK = len(BLOCKS)

    WCH = 4             # ko per w DMA chunk (k-pacing granularity)
    NCH = KO // WCH     # 4 chunks per block

    with tile.TileContext(nc) as tc:
        with (
            tc.tile_pool(name="xt_p", bufs=1) as xt_p,
            tc.tile_pool(name="w_p", bufs=1) as w_p,
            tc.tile_pool(name="o_p", bufs=4) as o_p,
            tc.tile_pool(name="wu_p", bufs=1) as wu_p,
            tc.tile_pool(name="ps", bufs=7, space="PSUM") as ps,
            tc.tile_pool(name="wu_ps_p", bufs=1, space="PSUM") as wu_ps_p,
        ):
            # --- PE warm-up: matmuls on scratch zeros, no DMA deps. They
            # run during the initial DMA wait and lift the HAM clock gate
            # toward 2.4GHz before the real stream starts.
            wu_lhs = wu_p.tile([P, P], COMPUTE_DT, tag="wu_lhs")
            wu_rhs = wu_p.tile([P, 512], COMPUTE_DT, tag="wu_rhs")
            nc.gpsimd.memset(wu_lhs[:], 0.0)
            nc.gpsimd.memset(wu_rhs[:], 0.0)
            wu_ps = wu_ps_p.tile([P, 512], mybir.dt.float32, tag="wu_ps")
            for i in range(N_WARMUP_MM):
                nc.tensor.matmul(wu_ps[:], wu_lhs[:], wu_rhs[:],
                                 start=(i == 0), stop=False,
                                 skip_group_check=True)

            # --- input DMAs, all on sync, in consumption order
            xt_t = [None] * mt
            w_c = [dict() for _ in range(NBK)]  # b -> ko -> (tile, off)

            def load_xt(mi):
                t = xt_p.tile([P, KO, P], COMPUTE_DT, tag=f"xt_m{mi}",
                              name=f"xt_m{mi}")
                nc.sync.dma_start(t[:], xt_d[mi])
                xt_t[mi] = t

            def load_w_chunk(b, c):
                c0, width = BLOCKS[b]
                ko0 = c * WCH
                t = w_p.tile([P, WCH, width], COMPUTE_DT, tag=f"w_b{b}_c{c}",
                             name=f"w_b{b}_c{c}")
                nc.sync.dma_start(t[:], w_d[:, ko0:ko0 + WCH, c0:c0 + width])
                for i in range(WCH):
                    w_c[b][ko0 + i] = (t, i)

            # diagonal head schedule needs one psum bank per m-tile
            diag_b0 = mt <= 6 and NBK > 1

            if diag_b0:
                # pairwise xt/w0 chunks so the (c,m) diagonal consumes in
                # exact arrival order
                for i in range(max(mt, NCH)):
                    if i < mt:
                        load_xt(i)
                    if i < NCH:
                        load_w_chunk(0, i)
            else:
                load_xt(0)
                for c in range(NCH):
                    load_w_chunk(0, c)
                for mi in range(1, mt):
                    load_xt(mi)
            for b in range(1, NBK):
                for c in range(NCH):
                    load_w_chunk(b, c)

            # --- compute ---
            def emit_out(b, m, psum_t, tag="o"):
                c0, width = BLOCKS[b]
                o_sb = o_p.tile([P, width], OUT_DT, tag=tag,
                                name=f"o_{b}_{m}")
                nc.vector.tensor_copy(o_sb[:], psum_t[:])
                nc.scalar.dma_start(
                    out_d[m * P:(m + 1) * P, c0:c0 + width], o_sb[:]
                )

            b_start = 0
            if diag_b0:
                # head block: (chunk, m) diagonal in data-arrival order.
                # 4-MM same-bank bursts let the PE do real work while the
                # rest of the head data streams in.
                b_start = 1
                w0, w0width = BLOCKS[0]
                psums0 = {
                    m: ps.tile([P, w0width], mybir.dt.float32, tag="psum",
                               name=f"psum_0_{m}")
                    for m in range(mt)
                }
                pairs = sorted(
                    ((c, m) for c in range(NCH) for m in range(mt)),
                    key=lambda cm: (cm[0] + cm[1], cm[0]),
                )
                for c, m in pairs:
                    for ko in range(c * WCH, (c + 1) * WCH):
                        w_t, wi = w_c[0][ko]
                        nc.tensor.matmul(
                            psums0[m][:],
                            xt_t[m][:, ko, :],
                            w_t[:, wi, :],
                            start=(ko == 0),
                            stop=(ko == KO - 1),
                        )
                    if c == NCH - 1:
                        emit_out(0, m, psums0[m])

            # remaining blocks: m-major, k-inner (dense same-bank
            # accumulation keeps the PE at the warm back-to-back rate);
            # per-chunk deps let each block's m0 pace with chunk arrival
            for b in range(b_start, NBK):
                c0, width = BLOCKS[b]
                for m in range(mt):
                    last = b == NBK - 1 and m == mt - 1
                    psum_t = ps.tile([P, width], mybir.dt.float32, tag="psum",
                                     name=f"psum_{b}_{m}")
                    for k in range(KO):
                        w_t, wi = w_c[b][k]
                        nc.tensor.matmul(
                            psum_t[:],
                            xt_t[m][:, k, :],
                            w_t[:, wi, :],
                            start=(k == 0),
                            stop=(k == KO - 1),
                        )
                    if last:
                        # split the final block so the tail DMAs are small
                        for h in range(2):
                            hw = width // 2
                            o_sb = o_p.tile([P, hw], OUT_DT,
                                            tag="olast", name=f"o_last{h}")
                            nc.vector.tensor_copy(
                                o_sb[:], psum_t[:, h * hw:(h + 1) * hw])
                            nc.scalar.dma_start(
                                out_d[m * P:(m + 1) * P,
                                      c0 + h * hw:c0 + (h + 1) * hw],
                                o_sb[:],
                            )
                    else:
                        emit_out(b, m, psum_t)

            # close the warm-up accumulation group
            nc.tensor.matmul(wu_ps[:], wu_lhs[:], wu_rhs[:],
                             start=False, stop=True, skip_group_check=True)

    nc.compile()
    return nc


def _swizzle_x(x_pad: np.ndarray, t_pad: int) -> np.ndarray:
    # [t_pad, K] f32 -> [mt, P, KO, P] bf16, xt[mi,p,ko,j] = X[mi*P+j, ko*P+p]
    mt = t_pad // P
    v = x_pad.reshape(mt, P, KO, P).transpose(0, 3, 2, 1)
    return np.ascontiguousarray(v.astype(NP_COMPUTE))


def _swizzle_w(w_g: np.ndarray) -> np.ndarray:
    # [K, N] f32 -> [P, KO, N], w[p,ko,j] = W[ko*P+p, j]
    v = w_g.reshape(KO, P, N).transpose(1, 0, 2)
    return np.ascontiguousarray(v.astype(NP_COMPUTE))


def _run(input, weight, tokens_per_expert, trace=False, **trace_kwargs):
    inp = np.ascontiguousarray(np.asarray(input), dtype=np.float32)
    wgt = np.ascontiguousarray(np.asarray(weight), dtype=np.float32)
    counts = np.asarray(tokens_per_expert).astype(np.int64)
    num_tokens, k = inp.shape
    assert k == K and wgt.shape == (G, K, N)
    # token group boundaries (matches searchsorted(cumsum, arange, 'right')),
    # clamped to the token range for safety on degenerate counts
    ends = np.minimum(np.cumsum(counts), num_tokens)
    starts = np.minimum(ends - counts, num_tokens)
    sizes = np.maximum(ends - starts, 0)

    t_pad = max(P, int(-(-max(int(sizes.max()), 1) // P)) * P)
    nc = _build(t_pad)

    in_maps = []
    for g in range(G):
        x_pad = np.zeros((t_pad, K), dtype=np.float32)
        x_pad[: sizes[g]] = inp[starts[g]:ends[g]]
        in_maps.append({"xt": _swizzle_x(x_pad, t_pad), "w": _swizzle_w(wgt[g])})

    res = bass_utils.run_bass_kernel_spmd(
        nc, in_maps, core_ids=list(range(G)), trace=trace, **trace_kwargs
    )

    # tokens not covered by any expert group get zero output (matches the
    # reference's masked accumulation)
    out = np.zeros((num_tokens, N), dtype=np.float32)
    for g in range(G):
        out[starts[g]:ends[g]] = res.results[g]["out"][: sizes[g]].astype(np.float32)
    return out, res


def kernel(input, weight, tokens_per_expert):
    out, _ = _run(input, weight, tokens_per_expert)
    return out



# revision 2
# speedup vs baseline: 1.0063x; 1.0063x over previous
"""AriaGroupedGEMM (MoE grouped GEMM) on 8 TRN2 NeuronCores.

Problem: input [4096, 2048] f32, weight [8, 2048, 2048] f32,
tokens_per_expert [8] int32 (tokens pre-sorted by expert).
out[i] = input[i] @ weight[expert_of(i)].

Strategy: expert-parallel. Core g owns expert g's weight and its token
group (boundaries computed on host from tokens_per_expert). Each core
runs a dense [T_pad, 2048] @ [2048, 2048] GEMM in bf16 (fp32 PSUM
accumulation): 256 matmuls of [128x128]@[128x512] = 54.6us of PE
streaming at the warm 2.4GHz back-to-back rate -- the compute floor.

This version is raw bacc (no TileContext) with manual semaphores:
 - inputs stream on the sync HWDGE ring in exact consumption order,
   every transfer fully contiguous (w is host-swizzled per n-block);
   each DMA has its own semaphore so data waits are exact.
 - thin N=128 warm-up matmuls keep the PE busy from t~0 so the HAM
   clock gate lifts to 2.4GHz while the first chunks stream in.
 - block 0 runs a wavefront (chunk, m) schedule that consumes data in
   arrival order; blocks 1-3 run dense m-major k-inner bursts.
 - 7 PSUM banks rotate across the 16 (block, m) groups; bank reuse is
   guarded by the DVE copy-done semaphore, slot reuse of the output
   staging tiles by per-out-DMA semaphores. Outputs leave on the
   scalar HWDGE ring as fully contiguous 128KB blocks.
 - no Tile epilogue: the ~8.5us per-semaphore reset storm at kernel
   end is replaced by bass's compact drain + range-clear.
"""
import sys
import functools

for _p in ("/opt/trn_rl_repo", "/root/.axon_site/_ro/trn_rl_repo"):
    if _p not in sys.path:
        sys.path.insert(0, _p)

import numpy as np
import ml_dtypes

import concourse.mybir as mybir
from concourse import bacc
from concourse import bass_utils

P = 128
K = 2048            # in_features (contraction)
N = 2048            # out_features
G = 8               # experts == cores
KO = K // P         # 16 k-subtiles
BW = 512            # n-block width (one PSUM bank of fp32)
NBLK = N // BW      # 4 n-blocks

COMPUTE_DT = mybir.dt.bfloat16
NP_COMPUTE = ml_dtypes.bfloat16
OUT_DT = mybir.dt.bfloat16      # psum(f32) -> bf16 on the way out; host upcasts

N_WARMUP_MM = 24    # thin N=128 warm-up matmuls (HAM ramp) before data lands
N_ROT = 7           # PSUM banks rotating over real groups (bank 8 = warmup)
N_OSB = 8           # output staging tiles in SBUF


@functools.lru_cache(maxsize=4)
def _build(t_pad: int):
    """Build + compile the per-core GEMM graph for token-pad t_pad."""
    mt = t_pad // P  # m tiles of 128 tokens

    nc = bacc.Bacc("TRN2", target_bir_lowering=False, debug=False)

    # host-swizzled DRAM layouts (fully contiguous per DMA):
    # xt[mi, p, ko*P + j] = X[mi*P + j, ko*P + p]
    # w[b, p, ko*BW + j]  = W[ko*P + p, b*BW + j]
    # out[b, t, j]        = OUT[t, b*BW + j]
    xt_d = nc.dram_tensor(
        "xt", [mt, P, KO * P], COMPUTE_DT, kind="ExternalInput"
    ).ap()
    w_d = nc.dram_tensor(
        "w", [NBLK, P, KO * BW], COMPUTE_DT, kind="ExternalInput"
    ).ap()
    out_d = nc.dram_tensor(
        "out", [NBLK, t_pad, BW], OUT_DT, kind="ExternalOutput"
    ).ap()

    # SBUF
    xt_sb = [nc.alloc_sbuf_tensor(f"xt_sb{m}", [P, KO * P], COMPUTE_DT).ap()
             for m in range(mt)]
    w_sb = [nc.alloc_sbuf_tensor(f"w_sb{b}", [P, KO * BW], COMPUTE_DT).ap()
            for b in range(NBLK)]
    o_sb = [nc.alloc_sbuf_tensor(f"o_sb{i}", [P, BW], OUT_DT).ap()
            for i in range(N_OSB)]
    wu_lhs = nc.alloc_sbuf_tensor("wu_lhs", [P, P], COMPUTE_DT).ap()
    wu_rhs = nc.alloc_sbuf_tensor("wu_rhs", [P, P], COMPUTE_DT).ap()

    # PSUM: 7 rotating banks for real groups + 1 warmup bank
    pk = [nc.alloc_psum_tensor(f"pk{j}", [P, BW], mybir.dt.float32).ap()
          for j in range(N_ROT)]
    wu_ps = nc.alloc_psum_tensor("wu_ps", [P, P], mybir.dt.float32).ap()

    NG = NBLK * mt  # real matmul groups

    # semaphores
    ws = nc.alloc_semaphore("ws")           # warmup operand memsets done
    pe_sem = nc.alloc_semaphore("pe_sem")   # PE group-final matmul done
    cp_sem = nc.alloc_semaphore("cp_sem")   # DVE psum->sbuf copy done
    od = [nc.alloc_semaphore(f"od{g}") for g in range(NG)]  # out DMA done

    # ---- gpsimd: warmup operand memsets
    nc.gpsimd.memset(wu_lhs, 0.0)
    nc.gpsimd.memset(wu_rhs, 0.0).then_inc(ws, 1)

    # ---- sync ring: all input DMAs in consumption order, one sem each
    dsem = []

    def dma_in(dst_ap, src_ap, tag):
        s = nc.alloc_semaphore(f"d{len(dsem)}_{tag}")
        nc.sync.dma_start(dst_ap, src_ap).then_inc(s, 16)
        dsem.append(s)
        return len(dsem) - 1

    def load_xt(m):
        return dma_in(xt_sb[m], xt_d[m], f"xt{m}")

    def load_w(b, k0, k1):
        return dma_in(w_sb[b][:, k0 * BW:k1 * BW],
                      w_d[b][:, k0 * BW:k1 * BW], f"w{b}_{k0}")

    # block-0 k-chunks: two small head chunks gate the first matmuls on
    # only ~768KB; the rest stream at 4-ko granularity
    B0_CHUNKS = [(0, 2), (2, 4), (4, 8), (8, 12), (12, 16)]
    BX_CHUNKS = [(0, 4), (4, 8), (8, 12), (12, 16)]

    d_xt = [None] * mt
    d_w0 = []           # b0 chunk sem ids
    d_wx = {}           # (b, ci) -> sem id for b >= 1

    if mt == 4:
        d_xt[0] = load_xt(0)
        d_w0.append(load_w(0, *B0_CHUNKS[0]))
        d_w0.append(load_w(0, *B0_CHUNKS[1]))
        d_xt[1] = load_xt(1)
        d_w0.append(load_w(0, *B0_CHUNKS[2]))
        d_xt[2] = load_xt(2)
        d_w0.append(load_w(0, *B0_CHUNKS[3]))
        d_xt[3] = load_xt(3)
        d_w0.append(load_w(0, *B0_CHUNKS[4]))
    else:
        for m in range(mt):
            d_xt[m] = load_xt(m)
        for c in B0_CHUNKS:
            d_w0.append(load_w(0, *c))
    for b in range(1, NBLK):
        for ci, c in enumerate(BX_CHUNKS):
            d_wx[(b, ci)] = load_w(b, *c)

    # ---- PE stream
    waited = set()

    def pe_wait(sem_id):
        if sem_id not in waited:
            nc.tensor.wait_ge(dsem[sem_id], 16)
            waited.add(sem_id)

    def mm(g, m, b, ko, start, stop):
        ins = nc.tensor.matmul(
            pk[g % N_ROT],
            xt_sb[m][:, ko * P:(ko + 1) * P],
            w_sb[b][:, ko * BW:(ko + 1) * BW],
            start=start,
            stop=stop,
        )
        if stop:
            ins.then_inc(pe_sem, 1)

    def bank_guard(g):
        # bank g%N_ROT was last used by group g-N_ROT; its DVE copy must
        # be done before we overwrite (PE write vs DVE read is fatal)
        if g >= N_ROT:
            nc.tensor.wait_ge(cp_sem, g - N_ROT + 1)

    # warmups: no data deps beyond the memsets; PE busy from ~t0
    nc.tensor.wait_ge(ws, 1)
    for _ in range(N_WARMUP_MM):
        nc.tensor.matmul(wu_ps, wu_lhs, wu_rhs, start=True, stop=True,
                         skip_group_check=True)

    # block 0: wavefront over (chunk, m) in data-arrival order
    if mt == 4:
        # (what's-ready -> emit) sequence matching the DMA interleave above
        waves = [
            ([d_xt[0], d_w0[0]], [(0, 0, 2)]),
            ([d_w0[1]],          [(0, 2, 4)]),
            ([d_xt[1]],          [(1, 0, 4)]),
            ([d_w0[2]],          [(0, 4, 8), (1, 4, 8)]),
            ([d_xt[2]],          [(2, 0, 8)]),
            ([d_w0[3]],          [(0, 8, 12), (1, 8, 12), (2, 8, 12)]),
            ([d_xt[3]],          [(3, 0, 12)]),
            ([d_w0[4]],          [(0, 12, 16), (1, 12, 16), (2, 12, 16),
                                  (3, 12, 16)]),
        ]
        for sems, spans in waves:
            for s in sems:
                pe_wait(s)
            for m, k0, k1 in spans:
                for ko in range(k0, k1):
                    mm(m, m, 0, ko, start=(ko == 0), stop=(ko == KO - 1))
    else:
        for m in range(mt):
            pe_wait(d_xt[m])
        for ci, (k0, k1) in enumerate(B0_CHUNKS):
            pe_wait(d_w0[ci])
            for m in range(mt):
                for ko in range(k0, k1):
                    mm(m, m, 0, ko, start=(ko == 0), stop=(ko == KO - 1))

    # blocks 1..: dense m-major k-inner bursts; chunk waits pace m0
    for b in range(1, NBLK):
        for m in range(mt):
            g = b * mt + m
            bank_guard(g)
            for ci, (k0, k1) in enumerate(BX_CHUNKS):
                if m == 0:
                    pe_wait(d_wx[(b, ci)])
                for ko in range(k0, k1):
                    mm(g, m, b, ko, start=(ko == 0), stop=(ko == KO - 1))

    # ---- DVE: psum -> sbuf staging (bf16)
    for g in range(NG):
        nc.vector.wait_ge(pe_sem, g + 1)
        if g >= N_OSB:
            # staging slot g%N_OSB was drained by out DMA g-N_OSB
            nc.vector.wait_ge(od[g - N_OSB], 16)
        nc.vector.tensor_copy(o_sb[g % N_OSB], pk[g % N_ROT]).then_inc(
            cp_sem, 1
        )

    # ---- scalar ring: output DMAs (each a contiguous 128KB block)
    for g in range(NG):
        b, m = divmod(g, mt)
        nc.scalar.wait_ge(cp_sem, g + 1)
        nc.scalar.dma_start(
            out_d[b][m * P:(m + 1) * P, :], o_sb[g % N_OSB]
        ).then_inc(od[g], 16)
    # make kernel end wait for the last outputs to land (earlier ones are
    # implied transitively by the DVE slot-reuse waits)
    for g in range(max(0, NG - N_OSB), NG):
        nc.scalar.wait_ge(od[g], 16)

    nc.compile()
    return nc


def _swizzle_x(x_pad: np.ndarray, t_pad: int) -> np.ndarray:
    # [t_pad, K] f32 -> [mt, P, KO*P] bf16, xt[mi,p,ko*P+j] = X[mi*P+j, ko*P+p]
    mt = t_pad // P
    v = x_pad.reshape(mt, P, KO, P).transpose(0, 3, 2, 1)
    return np.ascontiguousarray(
        v.astype(NP_COMPUTE).reshape(mt, P, KO * P))


def _swizzle_w(w_g: np.ndarray) -> np.ndarray:
    # [K, N] f32 -> [NBLK, P, KO*BW], w[b,p,ko*BW+j] = W[ko*P+p, b*BW+j]
    v = w_g.reshape(KO, P, NBLK, BW).transpose(2, 1, 0, 3)
    return np.ascontiguousarray(
        v.astype(NP_COMPUTE).reshape(NBLK, P, KO * BW))


def _run(input, weight, tokens_per_expert, trace=False, **trace_kwargs):
    inp = np.ascontiguousarray(np.asarray(input), dtype=np.float32)
    wgt = np.ascontiguousarray(np.asarray(weight), dtype=np.float32)
    counts = np.asarray(tokens_per_expert).astype(np.int64)
    num_tokens, k = inp.shape
    assert k == K and wgt.shape == (G, K, N)
    # token group boundaries (matches searchsorted(cumsum, arange, 'right')),
    # clamped to the token range for safety on degenerate counts
    ends = np.minimum(np.cumsum(counts), num_tokens)
    starts = np.minimum(ends - counts, num_tokens)
    sizes = np.maximum(ends - starts, 0)

    t_pad = max(P, int(-(-max(int(sizes.max()), 1) // P)) * P)
    nc = _build(t_pad)

    in_maps = []
    for g in range(G):
        x_pad = np.zeros((t_pad, K), dtype=np.float32)
        x_pad[: sizes[g]] = inp[starts[g]:ends[g]]
        in_maps.append({"xt": _swizzle_x(x_pad, t_pad), "w": _swizzle_w(wgt[g])})

    res = bass_utils.run_bass_kernel_spmd(
        nc, in_maps, core_ids=list(range(G)), trace=trace, **trace_kwargs
    )

    # tokens not covered by any expert group get zero output (matches the
    # reference's masked accumulation)
    out = np.zeros((num_tokens, N), dtype=np.float32)
    for g in range(G):
        blk = np.asarray(res.results[g]["out"])  # [NBLK, t_pad, BW]
        full = blk.transpose(1, 0, 2).reshape(t_pad, N)
        out[starts[g]:ends[g]] = full[: sizes[g]].astype(np.float32)
    return out, res


def kernel(input, weight, tokens_per_expert):
    out, _ = _run(input, weight, tokens_per_expert)
    return out


# revision 3
# speedup vs baseline: 1.0096x; 1.0032x over previous
"""AriaGroupedGEMM (MoE grouped GEMM) on 8 TRN2 NeuronCores.

Problem: input [4096, 2048] f32, weight [8, 2048, 2048] f32,
tokens_per_expert [8] int32 (tokens pre-sorted by expert).
out[i] = input[i] @ weight[expert_of(i)].

Strategy: expert-parallel. Core g owns expert g's weight and its token
group (boundaries computed on host from tokens_per_expert). Each core
runs a dense [T_pad, 2048] @ [2048, 2048] GEMM in bf16 (fp32 PSUM
accumulation): 256 matmuls of [128x128]@[128x512] = 54.6us of PE
streaming at the warm 2.4GHz back-to-back rate -- the compute floor.

Raw bacc (no TileContext), manual semaphores. Every HWDGE dma_start
occupies its sequencer ~0.65us (DIRECT2D trigger), so the input stream
is organized to keep the PE fed despite serialized triggering:

 - phase A interleaves n-blocks 0 and 1 at 4-ko chunk granularity
   across all 8 PSUM banks, so each arriving 512KB w-chunk unlocks
   16 matmuls (3.4us of PE work) -- PE saturates ~2 chunks in.
 - phase B (blocks 2 and 3) streams as two 2MB DMAs, prefetched far
   ahead, and runs dense m-major k-inner bursts.
 - the first matmul gates on only 512KB (half an x-tile + a 2-ko
   w-chunk); thin N=128 warm-up matmuls on uninitialized scratch keep
   the PE busy from the first instant so the HAM clock gate is lifted
   when real data lands.
 - outputs stage through 8 SBUF tiles and leave on the scalar HWDGE
   ring as contiguous 128KB blocks; per-DMA semaphores make every
   data wait exact.
"""
import sys
import functools

for _p in ("/opt/trn_rl_repo", "/root/.axon_site/_ro/trn_rl_repo"):
    if _p not in sys.path:
        sys.path.insert(0, _p)

import numpy as np
import ml_dtypes

import concourse.mybir as mybir
from concourse import bacc
from concourse import bass_utils

P = 128
K = 2048            # in_features (contraction)
N = 2048            # out_features
G = 8               # experts == cores
KO = K // P         # 16 k-subtiles
BW = 512            # n-block width (one PSUM bank of fp32)
NBLK = N // BW      # 4 n-blocks

COMPUTE_DT = mybir.dt.bfloat16
NP_COMPUTE = ml_dtypes.bfloat16
OUT_DT = mybir.dt.bfloat16      # psum(f32) -> bf16 on the way out; host upcasts

N_WARMUP_MM = 20    # thin N=128 warm-up matmuls (HAM ramp) before data lands
N_OSB = 8           # output staging tiles in SBUF


@functools.lru_cache(maxsize=4)
def _build(t_pad: int):
    """Build + compile the per-core GEMM graph for token-pad t_pad."""
    mt = t_pad // P  # m tiles of 128 tokens

    nc = bacc.Bacc("TRN2", target_bir_lowering=False, debug=False)

    # host-swizzled DRAM layouts (fully contiguous per DMA):
    # xt[mi, p, ko*P + j] = X[mi*P + j, ko*P + p]
    # w[b, p, ko*BW + j]  = W[ko*P + p, b*BW + j]
    # out[b, t, j]        = OUT[t, b*BW + j]
    xt_d = nc.dram_tensor(
        "xt", [mt, P, KO * P], COMPUTE_DT, kind="ExternalInput"
    ).ap()
    w_d = nc.dram_tensor(
        "w", [NBLK, P, KO * BW], COMPUTE_DT, kind="ExternalInput"
    ).ap()
    out_d = nc.dram_tensor(
        "out", [NBLK, t_pad, BW], OUT_DT, kind="ExternalOutput"
    ).ap()

    # SBUF
    xt_sb = [nc.alloc_sbuf_tensor(f"xt_sb{m}", [P, KO * P], COMPUTE_DT).ap()
             for m in range(mt)]
    w_sb = [nc.alloc_sbuf_tensor(f"w_sb{b}", [P, KO * BW], COMPUTE_DT).ap()
            for b in range(NBLK)]
    o_sb = [nc.alloc_sbuf_tensor(f"o_sb{i}", [P, BW], OUT_DT).ap()
            for i in range(N_OSB)]
    wu_lhs = nc.alloc_sbuf_tensor("wu_lhs", [P, P], COMPUTE_DT).ap()
    wu_rhs = nc.alloc_sbuf_tensor("wu_rhs", [P, P], COMPUTE_DT).ap()

    # PSUM: 8 banks; phase A owns all of them as (b, m) -> 4b+m for
    # b in {0,1}; phase B reuses bank (b-2)*4+m after its copy drains.
    # Warm-ups hit bank 7, which is clean because its first real tenant
    # (b1, m3) starts long after the warm-ups retire (PE is in-order).
    pk = [nc.alloc_psum_tensor(f"pk{j}", [P, BW], mybir.dt.float32).ap()
          for j in range(8)]
    wu_ps = pk[7][:, :P]

    NG = NBLK * mt  # real matmul groups

    pe_sem = nc.alloc_semaphore("pe_sem")   # PE group-final matmul done
    cp_sem = nc.alloc_semaphore("cp_sem")   # DVE psum->sbuf copy done
    od = [nc.alloc_semaphore(f"od{g}") for g in range(NG)]  # out DMA done

    # ---- sync ring: all input DMAs in consumption order, one sem each
    dsem = []

    def dma_in(dst_ap, src_ap, tag):
        s = nc.alloc_semaphore(f"d{len(dsem)}_{tag}")
        nc.sync.dma_start(dst_ap, src_ap).then_inc(s, 16)
        dsem.append(s)
        return len(dsem) - 1

    def load_xt(m, k0=0, k1=KO):
        return dma_in(xt_sb[m][:, k0 * P:k1 * P],
                      xt_d[m][:, k0 * P:k1 * P], f"xt{m}_{k0}")

    def load_w(b, k0, k1):
        return dma_in(w_sb[b][:, k0 * BW:k1 * BW],
                      w_d[b][:, k0 * BW:k1 * BW], f"w{b}_{k0}")

    # ---- PE stream helpers
    waited = set()

    def pe_wait(sem_id):
        if sem_id is not None and sem_id not in waited:
            nc.tensor.wait_ge(dsem[sem_id], 16)
            waited.add(sem_id)

    def mm(bank, m, b, ko):
        ins = nc.tensor.matmul(
            pk[bank],
            xt_sb[m][:, ko * P:(ko + 1) * P],
            w_sb[b][:, ko * BW:(ko + 1) * BW],
            start=(ko == 0),
            stop=(ko == KO - 1),
        )
        if ko == KO - 1:
            ins.then_inc(pe_sem, 1)

    # warm-ups: matmuls over uninitialized scratch (result never read);
    # no deps at all, so the PE is busy from the first instant
    for _ in range(N_WARMUP_MM):
        nc.tensor.matmul(wu_ps, wu_lhs, wu_rhs, start=True, stop=True,
                         skip_group_check=True)

    if mt == 4:
        # DMA order = PE consumption order (phase A interleaves b0/b1)
        d_xt0a = load_xt(0, 0, 8)
        d_w0a = load_w(0, 0, 2)
        d_xt0b = load_xt(0, 8, 16)
        d_w0b = load_w(0, 2, 4)
        d_xt = {1: load_xt(1), 0: None}
        d_w1a = load_w(1, 0, 4)
        d_xt[2] = load_xt(2)
        d_xt[3] = load_xt(3)
        d_wA = {}  # (b, ci) -> sem, chunks k4-7, k8-11, k12-15 for b0/b1
        for ci, (k0, k1) in enumerate([(4, 8), (8, 12), (12, 16)]):
            d_wA[(0, ci)] = load_w(0, k0, k1)
            d_wA[(1, ci)] = load_w(1, k0, k1)
        d_b2 = load_w(2, 0, 16)
        d_b3 = load_w(3, 0, 16)

        # phase A waves: (waits, [(m, b, ko-range)])
        waves = [
            ([d_xt0a, d_w0a], [(0, 0, 0, 2)]),
            ([d_w0b],         [(0, 0, 2, 4)]),
            ([d_xt[1]],       [(1, 0, 0, 4)]),
            ([d_w1a],         [(0, 1, 0, 4), (1, 1, 0, 4)]),
            ([d_xt0b],        []),
            ([d_xt[2]],       [(2, 0, 0, 4), (2, 1, 0, 4)]),
            ([d_xt[3]],       [(3, 0, 0, 4), (3, 1, 0, 4)]),
            ([d_wA[(0, 0)]],  [(m, 0, 4, 8) for m in range(4)]),
            ([d_wA[(1, 0)]],  [(m, 1, 4, 8) for m in range(4)]),
            ([d_wA[(0, 1)]],  [(m, 0, 8, 12) for m in range(4)]),
            ([d_wA[(1, 1)]],  [(m, 1, 8, 12) for m in range(4)]),
            ([d_wA[(0, 2)]],  [(m, 0, 12, 16) for m in range(4)]),
            ([d_wA[(1, 2)]],  [(m, 1, 12, 16) for m in range(4)]),
        ]
        for sems, spans in waves:
            for s in sems:
                pe_wait(s)
            for m, b, k0, k1 in spans:
                for ko in range(k0, k1):
                    mm(4 * b + m, m, b, ko)

        # phase B: dense m-major k-inner; bank (b-2)*4+m reused after
        # the copy of its phase-A tenant (group (b-2)*4+m) completed
        for b in (2, 3):
            for m in range(4):
                pe_wait(d_b2 if b == 2 else d_b3)
                bank = (b - 2) * 4 + m
                nc.tensor.wait_ge(cp_sem, bank + 1)
                for ko in range(KO):
                    mm(bank, m, b, ko)
    else:
        # generic fallback (never hit for the 512-tokens/expert case):
        # sequential blocks, chunk-paced, 7-bank rotation
        d_xt = [load_xt(m) for m in range(mt)]
        d_w = {}
        CH = [(0, 4), (4, 8), (8, 12), (12, 16)]
        for b in range(NBLK):
            for ci, (k0, k1) in enumerate(CH):
                d_w[(b, ci)] = load_w(b, k0, k1)
        for m in range(mt):
            pe_wait(d_xt[m])
        for b in range(NBLK):
            for m in range(mt):
                g = b * mt + m
                if g >= 7:
                    nc.tensor.wait_ge(cp_sem, g - 7 + 1)
                for ci, (k0, k1) in enumerate(CH):
                    pe_wait(d_w[(b, ci)])
                    for ko in range(k0, k1):
                        mm(g % 7, m, b, ko)

    # ---- DVE: psum -> sbuf staging (bf16); group order == stop order
    def group_bank(g):
        if mt == 4:
            return g if g < 8 else g - 8
        return g % 7

    for g in range(NG):
        nc.vector.wait_ge(pe_sem, g + 1)
        if g >= N_OSB:
            nc.vector.wait_ge(od[g - N_OSB], 16)
        nc.vector.tensor_copy(o_sb[g % N_OSB], pk[group_bank(g)]).then_inc(
            cp_sem, 1
        )

    # ---- scalar ring: output DMAs (each a contiguous 128KB block)
    for g in range(NG):
        b, m = divmod(g, mt)
        nc.scalar.wait_ge(cp_sem, g + 1)
        nc.scalar.dma_start(
            out_d[b][m * P:(m + 1) * P, :], o_sb[g % N_OSB]
        ).then_inc(od[g], 16)
    # make kernel end wait for the last outputs to land (earlier ones are
    # implied transitively by the DVE slot-reuse waits)
    for g in range(max(0, NG - N_OSB), NG):
        nc.scalar.wait_ge(od[g], 16)

    nc.compile()
    return nc


def _swizzle_x(x_pad: np.ndarray, t_pad: int) -> np.ndarray:
    # [t_pad, K] f32 -> [mt, P, KO*P] bf16, xt[mi,p,ko*P+j] = X[mi*P+j, ko*P+p]
    mt = t_pad // P
    v = x_pad.reshape(mt, P, KO, P).transpose(0, 3, 2, 1)
    return np.ascontiguousarray(
        v.astype(NP_COMPUTE).reshape(mt, P, KO * P))


def _swizzle_w(w_g: np.ndarray) -> np.ndarray:
    # [K, N] f32 -> [NBLK, P, KO*BW], w[b,p,ko*BW+j] = W[ko*P+p, b*BW+j]
    v = w_g.reshape(KO, P, NBLK, BW).transpose(2, 1, 0, 3)
    return np.ascontiguousarray(
        v.astype(NP_COMPUTE).reshape(NBLK, P, KO * BW))


def _run(input, weight, tokens_per_expert, trace=False, **trace_kwargs):
    inp = np.ascontiguousarray(np.asarray(input), dtype=np.float32)
    wgt = np.ascontiguousarray(np.asarray(weight), dtype=np.float32)
    counts = np.asarray(tokens_per_expert).astype(np.int64)
    num_tokens, k = inp.shape
    assert k == K and wgt.shape == (G, K, N)
    # token group boundaries (matches searchsorted(cumsum, arange, 'right')),
    # clamped to the token range for safety on degenerate counts
    ends = np.minimum(np.cumsum(counts), num_tokens)
    starts = np.minimum(ends - counts, num_tokens)
    sizes = np.maximum(ends - starts, 0)

    t_pad = max(P, int(-(-max(int(sizes.max()), 1) // P)) * P)
    nc = _build(t_pad)

    in_maps = []
    for g in range(G):
        x_pad = np.zeros((t_pad, K), dtype=np.float32)
        x_pad[: sizes[g]] = inp[starts[g]:ends[g]]
        in_maps.append({"xt": _swizzle_x(x_pad, t_pad), "w": _swizzle_w(wgt[g])})

    res = bass_utils.run_bass_kernel_spmd(
        nc, in_maps, core_ids=list(range(G)), trace=trace, **trace_kwargs
    )

    # tokens not covered by any expert group get zero output (matches the
    # reference's masked accumulation)
    out = np.zeros((num_tokens, N), dtype=np.float32)
    for g in range(G):
        blk = np.asarray(res.results[g]["out"])  # [NBLK, t_pad, BW]
        full = blk.transpose(1, 0, 2).reshape(t_pad, N)
        out[starts[g]:ends[g]] = full[: sizes[g]].astype(np.float32)
    return out, res


def kernel(input, weight, tokens_per_expert):
    out, _ = _run(input, weight, tokens_per_expert)
    return out


# revision 7
# speedup vs baseline: 1.0111x; 1.0015x over previous
"""AriaGroupedGEMM (MoE grouped GEMM) on 8 TRN2 NeuronCores.

Problem: input [4096, 2048] f32, weight [8, 2048, 2048] f32,
tokens_per_expert [8] int32 (tokens pre-sorted by expert).
out[i] = input[i] @ weight[expert_of(i)].

Strategy: expert-parallel. Core g owns expert g's weight and its token
group (boundaries computed on host from tokens_per_expert). Each core
runs a dense [T_pad, 2048] @ [2048, 2048] GEMM in bf16 (fp32 PSUM
accumulation): 256 matmuls of [128x128]@[128x512] = 54.6us of PE
streaming at the warm 2.4GHz back-to-back rate -- the compute floor.

Raw bacc (no TileContext), manual semaphores. Every HWDGE dma_start
occupies its sequencer ~0.65us (DIRECT2D trigger), so the input stream
is organized to keep the PE fed despite serialized triggering:

 - phase A interleaves n-blocks 0 and 1 at 4-ko chunk granularity
   across all 8 PSUM banks, so each arriving 512KB w-chunk unlocks
   16 matmuls (3.4us of PE work) -- PE saturates ~2 chunks in.
 - phase B (blocks 2 and 3) streams as two 2MB DMAs, prefetched far
   ahead, and runs dense m-major k-inner bursts.
 - the first matmul gates on only 512KB (half an x-tile + a 2-ko
   w-chunk); thin N=128 warm-up matmuls on uninitialized scratch keep
   the PE busy from the first instant so the HAM clock gate is lifted
   when real data lands.
 - outputs stage through 8 SBUF tiles and leave on the scalar HWDGE
   ring as contiguous 128KB blocks; per-DMA semaphores make every
   data wait exact.
"""
import sys
import functools

for _p in ("/opt/trn_rl_repo", "/root/.axon_site/_ro/trn_rl_repo"):
    if _p not in sys.path:
        sys.path.insert(0, _p)

import numpy as np
import ml_dtypes

import concourse.mybir as mybir
from concourse import bacc
from concourse import bass_utils

P = 128
K = 2048            # in_features (contraction)
N = 2048            # out_features
G = 8               # experts == cores
KO = K // P         # 16 k-subtiles
BW = 512            # n-block width (one PSUM bank of fp32)
NBLK = N // BW      # 4 n-blocks

COMPUTE_DT = mybir.dt.bfloat16
NP_COMPUTE = ml_dtypes.bfloat16
OUT_DT = mybir.dt.bfloat16      # psum(f32) -> bf16 on the way out; host upcasts

N_WARMUP_MM = 36    # thin N=128 warm-up matmuls (HAM ramp) before data lands
N_OSB = 8           # output staging tiles in SBUF


@functools.lru_cache(maxsize=4)
def _build(t_pad: int):
    """Build + compile the per-core GEMM graph for token-pad t_pad."""
    mt = t_pad // P  # m tiles of 128 tokens

    nc = bacc.Bacc("TRN2", target_bir_lowering=False, debug=False)

    # host-swizzled DRAM layouts (fully contiguous per DMA):
    # xt[mi, p, ko*P + j] = X[mi*P + j, ko*P + p]
    # w[b, p, ko*BW + j]  = W[ko*P + p, b*BW + j]
    # out[b, t, j]        = OUT[t, b*BW + j]
    xt_d = nc.dram_tensor(
        "xt", [mt, P, KO * P], COMPUTE_DT, kind="ExternalInput"
    ).ap()
    w_d = nc.dram_tensor(
        "w", [NBLK, P, KO * BW], COMPUTE_DT, kind="ExternalInput"
    ).ap()
    out_d = nc.dram_tensor(
        "out", [NBLK, t_pad, BW], OUT_DT, kind="ExternalOutput"
    ).ap()

    # SBUF
    xt_sb = [nc.alloc_sbuf_tensor(f"xt_sb{m}", [P, KO * P], COMPUTE_DT).ap()
             for m in range(mt)]
    w_sb = [nc.alloc_sbuf_tensor(f"w_sb{b}", [P, KO * BW], COMPUTE_DT).ap()
            for b in range(NBLK)]
    o_sb = [nc.alloc_sbuf_tensor(f"o_sb{i}", [P, BW], OUT_DT).ap()
            for i in range(N_OSB)]
    wu_lhs = nc.alloc_sbuf_tensor("wu_lhs", [P, P], COMPUTE_DT).ap()
    wu_rhs = nc.alloc_sbuf_tensor("wu_rhs", [P, P], COMPUTE_DT).ap()

    # PSUM: 8 banks; phase A owns all of them as (b, m) -> 4b+m for
    # b in {0,1}; phase B reuses bank (b-2)*4+m after its copy drains.
    # Warm-ups hit bank 7, which is clean because its first real tenant
    # (b1, m3) starts long after the warm-ups retire (PE is in-order).
    pk = [nc.alloc_psum_tensor(f"pk{j}", [P, BW], mybir.dt.float32).ap()
          for j in range(8)]
    wu_ps = pk[7][:, :P]

    NG = NBLK * mt  # real matmul groups

    pe_sem = nc.alloc_semaphore("pe_sem")   # PE group-final matmul done
    cp_sem = nc.alloc_semaphore("cp_sem")   # DVE psum->sbuf copy done
    od = [nc.alloc_semaphore(f"od{g}") for g in range(NG)]  # out DMA done

    # ---- input DMAs in consumption order, one sem each; the two gating
    # transfers trigger concurrently on both HWDGE rings (each dma_start
    # occupies its sequencer ~0.65us)
    dsem = []

    def dma_in(dst_ap, src_ap, tag, ring=None):
        s = nc.alloc_semaphore(f"d{len(dsem)}_{tag}")
        (ring or nc.sync).dma_start(dst_ap, src_ap).then_inc(s, 16)
        dsem.append(s)
        return len(dsem) - 1

    def load_xt(m, k0=0, k1=KO, ring=None):
        return dma_in(xt_sb[m][:, k0 * P:k1 * P],
                      xt_d[m][:, k0 * P:k1 * P], f"xt{m}_{k0}", ring)

    def load_w(b, k0, k1, ring=None):
        return dma_in(w_sb[b][:, k0 * BW:k1 * BW],
                      w_d[b][:, k0 * BW:k1 * BW], f"w{b}_{k0}", ring)

    # ---- PE stream helpers
    waited = set()

    def pe_wait(sem_id):
        if sem_id is not None and sem_id not in waited:
            nc.tensor.wait_ge(dsem[sem_id], 16)
            waited.add(sem_id)

    def mm(bank, m, b, ko):
        ins = nc.tensor.matmul(
            pk[bank],
            xt_sb[m][:, ko * P:(ko + 1) * P],
            w_sb[b][:, ko * BW:(ko + 1) * BW],
            start=(ko == 0),
            stop=(ko == KO - 1),
        )
        if ko == KO - 1:
            ins.then_inc(pe_sem, 1)

    # warm-ups: matmuls over uninitialized scratch (result never read);
    # no deps at all, so the PE is busy from the first instant
    for _ in range(N_WARMUP_MM):
        nc.tensor.matmul(wu_ps, wu_lhs, wu_rhs, start=True, stop=True,
                         skip_group_check=True)

    if mt == 4:
        # DMA order = PE consumption order (phase A interleaves b0/b1).
        # The two gating transfers go out concurrently: xt0's first half
        # on sync, b0's first 2-ko chunk (and xt0's second half) on the
        # scalar ring, whose output section only activates much later.
        d_xt0a = load_xt(0, 0, 8)
        d_w0a = load_w(0, 0, 2, ring=nc.scalar)
        d_xt0b = load_xt(0, 8, 16, ring=nc.scalar)
        d_w0b = load_w(0, 2, 4)
        d_xt = {1: load_xt(1), 0: None}
        d_w1a = load_w(1, 0, 4)
        d_xt[2] = load_xt(2)
        d_xt[3] = load_xt(3)
        d_wA = {}  # (b, ci) -> sem, chunks k4-7, k8-11, k12-15 for b0/b1
        for ci, (k0, k1) in enumerate([(4, 8), (8, 12), (12, 16)]):
            d_wA[(0, ci)] = load_w(0, k0, k1)
            d_wA[(1, ci)] = load_w(1, k0, k1)
        d_b2 = load_w(2, 0, 16)
        d_b3 = load_w(3, 0, 16)

        # phase A waves: (waits, [(m, b, ko-range)])
        waves = [
            ([d_xt0a, d_w0a], [(0, 0, 0, 2)]),
            ([d_w0b],         [(0, 0, 2, 4)]),
            ([d_xt[1]],       [(1, 0, 0, 4)]),
            ([d_w1a],         [(0, 1, 0, 4), (1, 1, 0, 4)]),
            ([d_xt0b],        []),
            ([d_xt[2]],       [(2, 0, 0, 4), (2, 1, 0, 4)]),
            ([d_xt[3]],       [(3, 0, 0, 4), (3, 1, 0, 4)]),
            ([d_wA[(0, 0)]],  [(m, 0, 4, 8) for m in range(4)]),
            ([d_wA[(1, 0)]],  [(m, 1, 4, 8) for m in range(4)]),
            ([d_wA[(0, 1)]],  [(m, 0, 8, 12) for m in range(4)]),
            ([d_wA[(1, 1)]],  [(m, 1, 8, 12) for m in range(4)]),
            ([d_wA[(0, 2)]],  [(m, 0, 12, 16) for m in range(4)]),
            ([d_wA[(1, 2)]],  [(m, 1, 12, 16) for m in range(4)]),
        ]
        for sems, spans in waves:
            for s in sems:
                pe_wait(s)
            for m, b, k0, k1 in spans:
                for ko in range(k0, k1):
                    mm(4 * b + m, m, b, ko)

        # phase B: dense m-major k-inner; bank (b-2)*4+m reused after
        # the copy of its phase-A tenant (group (b-2)*4+m) completed
        for b in (2, 3):
            for m in range(4):
                pe_wait(d_b2 if b == 2 else d_b3)
                bank = (b - 2) * 4 + m
                nc.tensor.wait_ge(cp_sem, bank + 1)
                for ko in range(KO):
                    mm(bank, m, b, ko)
    else:
        # generic fallback (never hit for the 512-tokens/expert case):
        # sequential blocks, chunk-paced, 7-bank rotation
        d_xt = [load_xt(m) for m in range(mt)]
        d_w = {}
        CH = [(0, 4), (4, 8), (8, 12), (12, 16)]
        for b in range(NBLK):
            for ci, (k0, k1) in enumerate(CH):
                d_w[(b, ci)] = load_w(b, k0, k1)
        for m in range(mt):
            pe_wait(d_xt[m])
        for b in range(NBLK):
            for m in range(mt):
                g = b * mt + m
                if g >= 7:
                    nc.tensor.wait_ge(cp_sem, g - 7 + 1)
                for ci, (k0, k1) in enumerate(CH):
                    pe_wait(d_w[(b, ci)])
                    for ko in range(k0, k1):
                        mm(g % 7, m, b, ko)

    # ---- DVE: psum -> sbuf staging (bf16); group order == stop order
    def group_bank(g):
        if mt == 4:
            return g if g < 8 else g - 8
        return g % 7

    for g in range(NG):
        nc.vector.wait_ge(pe_sem, g + 1)
        if g >= N_OSB:
            nc.vector.wait_ge(od[g - N_OSB], 16)
        nc.vector.tensor_copy(o_sb[g % N_OSB], pk[group_bank(g)]).then_inc(
            cp_sem, 1
        )

    # ---- scalar ring: output DMAs (each a contiguous 128KB block)
    for g in range(NG):
        b, m = divmod(g, mt)
        nc.scalar.wait_ge(cp_sem, g + 1)
        nc.scalar.dma_start(
            out_d[b][m * P:(m + 1) * P, :], o_sb[g % N_OSB]
        ).then_inc(od[g], 16)
    # no end-of-kernel waits on the output DMAs: the fixed walrus NEFF
    # epilogue (per-semaphore resets, ~6.5us after the final barrier) runs
    # long past the last output's completion receipt, so the outputs are
    # guaranteed landed before the NEFF can finish either way

    nc.compile()
    return nc


def _swizzle_x(x_pad: np.ndarray, t_pad: int) -> np.ndarray:
    # [t_pad, K] f32 -> [mt, P, KO*P] bf16, xt[mi,p,ko*P+j] = X[mi*P+j, ko*P+p]
    mt = t_pad // P
    v = x_pad.reshape(mt, P, KO, P).transpose(0, 3, 2, 1)
    return np.ascontiguousarray(
        v.astype(NP_COMPUTE).reshape(mt, P, KO * P))


def _swizzle_w(w_g: np.ndarray) -> np.ndarray:
    # [K, N] f32 -> [NBLK, P, KO*BW], w[b,p,ko*BW+j] = W[ko*P+p, b*BW+j]
    v = w_g.reshape(KO, P, NBLK, BW).transpose(2, 1, 0, 3)
    return np.ascontiguousarray(
        v.astype(NP_COMPUTE).reshape(NBLK, P, KO * BW))


def _run(input, weight, tokens_per_expert, trace=False, **trace_kwargs):
    inp = np.ascontiguousarray(np.asarray(input), dtype=np.float32)
    wgt = np.ascontiguousarray(np.asarray(weight), dtype=np.float32)
    counts = np.asarray(tokens_per_expert).astype(np.int64)
    num_tokens, k = inp.shape
    assert k == K and wgt.shape == (G, K, N)
    # token group boundaries (matches searchsorted(cumsum, arange, 'right')),
    # clamped to the token range for safety on degenerate counts
    ends = np.minimum(np.cumsum(counts), num_tokens)
    starts = np.minimum(ends - counts, num_tokens)
    sizes = np.maximum(ends - starts, 0)

    t_pad = max(P, int(-(-max(int(sizes.max()), 1) // P)) * P)
    nc = _build(t_pad)

    in_maps = []
    for g in range(G):
        x_pad = np.zeros((t_pad, K), dtype=np.float32)
        x_pad[: sizes[g]] = inp[starts[g]:ends[g]]
        in_maps.append({"xt": _swizzle_x(x_pad, t_pad), "w": _swizzle_w(wgt[g])})

    res = bass_utils.run_bass_kernel_spmd(
        nc, in_maps, core_ids=list(range(G)), trace=trace, **trace_kwargs
    )

    # tokens not covered by any expert group get zero output (matches the
    # reference's masked accumulation)
    out = np.zeros((num_tokens, N), dtype=np.float32)
    for g in range(G):
        blk = np.asarray(res.results[g]["out"])  # [NBLK, t_pad, BW]
        full = blk.transpose(1, 0, 2).reshape(t_pad, N)
        out[starts[g]:ends[g]] = full[: sizes[g]].astype(np.float32)
    return out, res


def kernel(input, weight, tokens_per_expert):
    out, _ = _run(input, weight, tokens_per_expert)
    return out
